# revision 61
# baseline (speedup 1.0000x reference)
"""AttnBlock (GroupNorm + single-head self-attention + proj + residual) for
Trainium2, SPMD over 8 NeuronCores, fp8 DoubleRow edition.

Problem: hidden_states [4, 64, 64, 512]; per batch element b: x = GN(h_b)
(32 groups over (H, W, chans)), q/k/v = x@W + b, attn = softmax(q k^T / sqrt
(sqrt C)), out = (attn @ v) @ Wp + bp + residual.

Sharding: 8 cores = 4 batch elements x 2 query-halves. Each core receives the
full image of its batch element (for GN stats and K/V) plus its half of the
rows (queries + residual), and produces its [2048, 512] output slice. Cores
are fully independent - no collectives.

Per-core dataflow (all heavy matmuls in fp8e4 DoubleRow perf mode, which
contracts two 128-deep k-tiles per pass at 2x rate; rel-err budget is 2e-2
and the full-fp8 pipeline measures ~1e-2 in simulation):
  1. x arrives host-transposed and host-cast to fp8 as xT [c, n]. GN stats
     via DVE bn_stats on the fp8 values; group reduce/broadcast via tiny mask
     matmuls; GN affine is folded into the QKV weights (W <- a*W,
     bias <- b^T W + bias), so x is never normalized explicitly.
  2. K^T[c_out, n], Q^T[c_out, q], V[n, c_out] fp8 GEMMs (weights host-cast
     to bf16, scaled by a on device, cast to fp8). PSUM -> fp8 SBUF casts
     are split across DVE and ScalarE (Identity with per-partition bias) -
     they are the phase-2 bottleneck. V's bias contribution rides in the
     proj bias instead (softmax weights sum to 1, so E@(V+1*bv2)/d @ Wp =
     attnV@Wp + bv2@Wp). V stays resident in SBUF - no DRAM spill.
  3. attention, software-pipelined over 8 query blocks of 256: per key-tile
     pair S^T = K^T-pair @ Q^T-pair (DoubleRow), exp via ScalarE over 4-key-
     tile chunks with the logit scale and a constant shift -C_SHIFT folded in
     (keeps E <= ~130 < 240 = TRN fp8e4 max), E^T in fp8; softmax denominator
     d = ones^T E^T on the PE; O^T = V-pair @ E^T accumulated over key pairs.
     The PE interleaves S(qb) with PV(qb-1) so it never waits on the ScalarE
     exp stream; proj(qb-2) trails two blocks behind. The d chain is front-
     loaded so the 1/d DRAM roundtrip overlaps the cc chains.
  4. O^T is normalized by 1/d during the PSUM->SBUF fp8 cast (1/d broadcast
     across partitions via a tiny DRAM round trip), so Y = O^T @ Wp needs no
     further scaling; bp (+ bv2 @ Wp) is seeded into the proj PSUM with a
     ones-column matmul; epilogue adds the residual rows and streams out.
"""

import math

import numpy as np

import concourse.bass as bass
import concourse.tile as tile
from concourse import mybir

F32 = mybir.dt.float32
BF16 = mybir.dt.bfloat16
F8 = mybir.dt.float8e4
AF = mybir.ActivationFunctionType
ALU = mybir.AluOpType
DR = mybir.MatmulPerfMode.DoubleRow

B, HH, WW, C = 4, 64, 64, 512
N = HH * WW            # 4096 tokens per image
NQ = N // 2            # 2048 queries per core
G = 32                 # groups
GS = C // G            # 16 channels per group
EPS = 1e-6
SCALE2 = 1.0 / math.sqrt(float(C))   # (1/C^0.25)^2, applied to logits
C_SHIFT = 2.0          # global logit shift so exp() stays in fp8 range
P = 128
CT = C // P            # 4 channel tiles
NT_KV = N // P         # 32 key tiles (full image)
QB = 256               # query block
QBN = NQ // QB         # 8 q-blocks
KP = NT_KV // 2        # 16 key-tile pairs


def _apply_drain_patch():
    """This container's walrus rejects instructions with more than a couple of
    sync-waits; the TileContext end-of-kernel drain accumulates one wait per
    live processor. Redistribute them across SP nops (one wait each)."""
    import concourse.tile as tile_mod

    if getattr(tile_mod.TileContext, "_drain_patch_applied", False):
        return

    def _drain_and_barrier(self, tick_clock, wait_clock):
        from concourse.vector_clock import ScopedClock

        nc = self.nc
        drain_inst = nc.sync.drain()
        wait_clock.add_sem_waits(
            drain_inst.ins, ScopedClock({None: tick_clock.global_clock})
        )
        si = drain_inst.ins.sync_info
        waits = list(si.on_wait or []) if si else []
        if len(waits) > 1:
            drain_inst.ins.sync_info = mybir.SyncInfo(
                on_wait=waits[:1], on_update=list(si.on_update or [])
            )
            for i in range(1, len(waits)):
                nop = nc.sync.nop()
                nop.ins.sync_info = mybir.SyncInfo(
                    on_wait=waits[i : i + 1], on_update=[]
                )
        nc.all_engine_barrier()
        popped = nc._tile_sem_poison_stack.pop()
        assert popped is self._sem_poison
        nc.clear_and_free_semaphores(list(self.sems.allocated().values()))
        nc.all_engine_barrier()

    tile_mod.TileContext._drain_and_barrier = _drain_and_barrier
    tile_mod.TileContext._drain_patch_applied = True


def _split_excess_waits(nc, max_waits=1):
    """This walrus build accepts only a very small number of sync-wait
    commands per instruction (a fused Matmult rejects even 2). Hoist excess
    waits onto same-engine nops inserted immediately before the owner."""
    fn = nc.m.functions[0]
    for block in list(fn.blocks):
        insts = block.instructions
        new = []
        for inst in insts:
            si = inst.sync_info
            waits = list(si.on_wait or []) if si else []
            if len(waits) > max_waits and inst.engine in nc.engines:
                inst.sync_info = mybir.SyncInfo(
                    on_wait=waits[-max_waits:],
                    on_update=list(si.on_update or []),
                )
                excess = waits[:-max_waits]
                for j in range(0, len(excess), max_waits):
                    nop = nc.engines[inst.engine].nop(nofuse=True)
                    ni = nop.ins
                    # the builder appended it to the current bb; pull it out
                    removed = False
                    for b2 in fn.blocks:
                        l2 = b2.instructions
                        if l2 and l2[-1] is ni:
                            l2.pop()
                            removed = True
                            break
                    assert removed, "could not relocate wait-carrier nop"
                    ni.sync_info = mybir.SyncInfo(
                        on_wait=excess[j : j + max_waits], on_update=[]
                    )
                    new.append(ni)
            new.append(inst)
        block.instructions[:] = new


def build_nc(iters=1, debug=False):
    _apply_drain_patch()
    nc = bass.Bass(enable_partition_id=False)

    def param(name, shape, is_out=False, dtype=F32):
        h = nc.declare_dram_parameter(name, shape, dtype, isOutput=is_out)
        return h[tuple(slice(None) for _ in shape)]

    xT = param("xT", [C, N], dtype=F8)      # host-transposed + fp8-cast
    x_res = param("x_res", [NQ, C])         # residual rows (row-major, fp32)
    gmask = param("gmask", [P, G // CT])    # gmask[p, j] = (p//GS == j)
    gmask2 = param("gmask2", [G // CT, P])  # transpose of gmask
    gns_p = param("gns_p", [P, CT])         # gn_scale in partition layout
    gnb_p = param("gnb_p", [P, CT])         # gn_bias in partition layout
    wq = param("wq", [C, C], dtype=BF16)
    wk = param("wk", [C, C], dtype=BF16)
    wv = param("wv", [C, C], dtype=BF16)
    wp = param("wp", [C, C], dtype=F8)      # no GN fold needed -> direct fp8
    bq = param("bq", [C])
    bk = param("bk", [C])
    bv = param("bv", [C])
    bp = param("bp", [C])
    out = param("out", [NQ, C], is_out=True)
    if debug:
        kT_dbg = param("kT_dbg", [P, CT, N], is_out=True, dtype=F8)
        qT_dbg = param("qT_dbg", [P, CT, NQ], is_out=True, dtype=F8)
        v_dbg = param("v_dbg", [P, NT_KV, C], is_out=True, dtype=F8)
        e_dbg = param("e_dbg", [P, NT_KV, QB], is_out=True, dtype=F8)
        d_dbg = param("d_dbg", [1, QB], is_out=True)
        o_dbg = param("o_dbg", [P, CT, QB], is_out=True, dtype=F8)
        oraw_dbg = param("oraw_dbg", [P, CT, QB], is_out=True)

    def bcast_ap(vec_ap, parts):
        # [C]-shaped DRAM vector -> [parts, C] partition-stride-0 DMA source
        return bass.AP(
            tensor=vec_ap.tensor,
            offset=vec_ap.offset,
            ap=[[0, parts]] + [list(d) for d in vec_ap.ap],
        )

    with tile.TileContext(nc) as tc:

        def emit_body(sfx):
            # ---- long-lived pools ----
            dscratch = tc.alloc_tile_pool(name=f"dscratch{sfx}", bufs=1, space="DRAM")
            bias_dram = dscratch.tile([3, C], F32, name="bias_dram")
            rd_dram = dscratch.tile([QBN, QB], F32, name="rd_dram")
            consts = tc.alloc_tile_pool(name=f"consts{sfx}", bufs=1, side="left")
            stream = tc.alloc_tile_pool(name=f"stream{sfx}", bufs=3, side="left")
            small = tc.alloc_tile_pool(name=f"small{sfx}", bufs=1, side="left")

            # fp8 ones [128, 2, 1] for the softmax-denominator matmuls;
            # bf16 ones row [1, 128] for the bias-seed matmuls.
            # pair stride must be even + 16B aligned for dual-fp8 LDWEIGHTS
            ones8_t = consts.tile([P, 2, 16], F8, name="ones8_t")
            nc.vector.memset(ones8_t, 1.0)
            ones8 = ones8_t[:, :, 0:1]
            ones_bf = consts.tile([1, P], BF16, name="ones_bf")
            nc.vector.memset(ones_bf, 1.0)
            bp_f = consts.tile([1, C], F32, name="bp_f")
            nc.gpsimd.dma_start(bp_f, bp[None, :])
            # bp_row = bf16(bp + bv2 @ Wp): since softmax weights sum to 1,
            # the V bias contributes exactly bv2 @ Wp to the proj output, so
            # it is folded here instead of being added to V. Filled in after
            # the folded biases exist (see phase 2).
            bp_row = consts.tile([1, C], BF16, name="bp_row")
            negc = consts.tile([P, 1], F32, name="negc")
            nc.vector.memset(negc, -C_SHIFT)

            # per-channel norm scale/bias in partition layout
            a_p = small.tile([P, CT], F32, name="a_p")
            b_p = small.tile([P, CT], F32, name="b_p")
            b_pr = small.tile([P, CT], BF16, name="b_pr")

            # ---- phase 1: load x^T (fp8), stats via DVE bn_stats ----
            xkvT, free_xkvT = tc.tile([P, CT, N], F8, name="xkvT", side="right")
            p1tmp = tc.alloc_tile_pool(name=f"p1tmp{sfx}", bufs=1, side="left")
            eps_t = p1tmp.tile([P, 1], F32, name="eps_t")
            nc.vector.memset(eps_t, EPS)
            gmask_s = p1tmp.tile([P, G // CT], F32, name="gmask_s")
            nc.gpsimd.dma_start(gmask_s, gmask)
            gmask2_s = p1tmp.tile([G // CT, P], F32, name="gmask2_s")
            nc.gpsimd.dma_start(gmask2_s, gmask2)
            gns_s = p1tmp.tile([P, CT], F32, name="gns_s")
            nc.gpsimd.dma_start(gns_s, gns_p)
            gnb_s = p1tmp.tile([P, CT], F32, name="gnb_s")
            nc.gpsimd.dma_start(gnb_s, gnb_p)
            stats_p = p1tmp.tile([P, 2 * CT], F32, name="stats_p")
            NBCH = N // 512
            bnst = p1tmp.tile([P, NBCH, 6], F32, name="bnst")
            mv = p1tmp.tile([P, 2], F32, name="mv")

            xTv = xT.rearrange("(ko ki) n -> ki ko n", ki=P)
            NPC = 2  # DMA pieces per channel tile
            PW = N // NPC
            # ScalarE's stats data first (its leg is longest): ct3 both
            # halves, then ct2 (DVE half first), then ct0/ct1 for DVE
            for ct, pc in ((3, 0), (3, 1), (2, 0), (2, 1), (0, 0), (0, 1), (1, 0), (1, 1)):
                w0 = pc * PW
                nc.sync.dma_start(
                    xkvT[:, ct, w0 : w0 + PW], xTv[:, ct, w0 : w0 + PW]
                )
            # per-channel sums: DVE bn_stats for ct 0-1 + ct2 first half,
            # ScalarE Copy/Square with free-dim accumulators for the rest
            acc_t = p1tmp.tile([P, 3, 2], F32, name="acc_t")  # [piece, sum/sq]
            sc_pieces = [(2, N // 2, N), (3, 0, N // 2), (3, N // 2, N)]
            for i, (ct, w0, w1) in enumerate(sc_pieces):
                xs = xkvT[:, ct, w0:w1]
                dump = p1tmp.tile([P, N // 2], F8, name="sq_dump", bufs=2)
                nc.scalar.activation(
                    dump[:, : w1 - w0], xs, AF.Copy,
                    accum_out=acc_t[:, i, 0:1],
                )
                dump2 = p1tmp.tile([P, N // 2], F8, name="sq_dump", bufs=2)
                nc.scalar.activation(
                    dump2[:, : w1 - w0], xs, AF.Square,
                    accum_out=acc_t[:, i, 1:2],
                )
            mv2 = p1tmp.tile([P, 2], F32, name="mv2")
            bnst2 = p1tmp.tile([P, NBCH // 2, 6], F32, name="bnst2")
            for ct in (2, 0, 1):
                nch = NBCH if ct < 2 else NBCH // 2
                xv = xkvT[:, ct, :].rearrange("p (s f) -> p s f", f=512)
                bn = bnst if ct < 2 else bnst2
                for s in range(nch):
                    nc.vector.bn_stats(bn[:, s, :], xv[:, s, :])
                m = mv if ct < 2 else mv2
                nc.vector.bn_aggr(m, bn)
                cnt = float(N) if ct < 2 else float(N // 2)
                # sum = mean*cnt ; sumsq = (var + mean^2)*cnt
                nc.vector.tensor_scalar_mul(
                    stats_p[:, ct : ct + 1], m[:, 0:1], cnt
                )
                nc.vector.tensor_mul(
                    stats_p[:, CT + ct : CT + ct + 1], m[:, 0:1], m[:, 0:1]
                )
                nc.vector.tensor_tensor(
                    stats_p[:, CT + ct : CT + ct + 1],
                    m[:, 1:2], stats_p[:, CT + ct : CT + ct + 1], ALU.add,
                )
                nc.vector.tensor_scalar_mul(
                    stats_p[:, CT + ct : CT + ct + 1],
                    stats_p[:, CT + ct : CT + ct + 1], cnt,
                )
            # ct2 = DVE first half + ScalarE second half; ct3 = ScalarE both
            nc.vector.tensor_tensor(
                stats_p[:, 2:3], stats_p[:, 2:3], acc_t[:, 0, 0:1], ALU.add
            )
            nc.vector.tensor_tensor(
                stats_p[:, CT + 2 : CT + 3],
                stats_p[:, CT + 2 : CT + 3], acc_t[:, 0, 1:2], ALU.add,
            )
            nc.vector.tensor_tensor(
                stats_p[:, 3:4], acc_t[:, 1, 0:1], acc_t[:, 2, 0:1], ALU.add
            )
            nc.vector.tensor_tensor(
                stats_p[:, CT + 3 : CT + 4],
                acc_t[:, 1, 1:2], acc_t[:, 2, 1:2], ALU.add,
            )

            # ---- phase 1b: group reduce/broadcast via tiny mask matmuls ----
            ps1 = tc.alloc_tile_pool(name=f"ps1{sfx}", bufs=1, space="PSUM")
            ps_g = ps1.tile([G // CT, 2 * CT], F32, name="ps_g")
            nc.tensor.matmul(ps_g, lhsT=gmask_s, rhs=stats_p, start=True, stop=True)
            gvals = p1tmp.tile([G // CT, 2 * CT], F32, name="gvals")
            nc.vector.tensor_copy(gvals, ps_g)
            ps_b = ps1.tile([P, 2 * CT], F32, name="ps_b")
            nc.tensor.matmul(ps_b, lhsT=gmask2_s, rhs=gvals, start=True, stop=True)
            sums_b = p1tmp.tile([P, 2 * CT], F32, name="sums_b")
            inv_cnt = 1.0 / float(N * GS)
            nc.vector.tensor_scalar_mul(sums_b, ps_b, inv_cnt)
            mean_p = sums_b[:, 0:CT]       # E[x] per channel's group
            e2_p = sums_b[:, CT : 2 * CT]  # E[x^2]
            var_p = p1tmp.tile([P, CT], F32, name="var_p")
            nc.vector.tensor_mul(var_p, mean_p, mean_p)
            nc.vector.tensor_tensor(var_p, e2_p, var_p, ALU.subtract)
            # rstd = 1/sqrt(var + eps); a = rstd*gamma; b = beta - mean*a
            nc.scalar.activation(var_p, var_p, AF.Sqrt, bias=eps_t)
            nc.vector.reciprocal(var_p, var_p)
            nc.vector.tensor_mul(a_p, var_p, gns_s)
            nc.vector.tensor_mul(b_p, mean_p, a_p)
            nc.vector.tensor_tensor(b_p, gnb_s, b_p, ALU.subtract)
            nc.vector.tensor_copy(b_pr, b_p)
            ps1.release()
            p1tmp.release()

            # ---- phase 2: fold GN affine into weights, K/Q/V fp8 GEMMs ----
            kT, free_kT = tc.tile([P, CT, N], F8, name="kT", side="left")
            qT, free_qT = tc.tile([P, CT, NQ], F8, name="qT", side="left")
            v_sb, free_v = tc.tile([P, NT_KV, C], F8, name="v_sb", side="left")
            wf_pool = tc.alloc_tile_pool(name=f"wf_pool{sfx}", bufs=1, side="left")
            w_pool = tc.alloc_tile_pool(name=f"w_pool{sfx}", bufs=1, side="left")

            def load_w(name, w, dt):
                # sync (SP) queue so these stay behind the x pieces on the
                # serial DMA pipe - the Pool queue would jump the line
                t = w_pool.tile([P, CT, C], dt, name=name)
                nc.sync.dma_start(t, w.rearrange("(ko ki) n -> ki ko n", ki=P))
                return t

            wk_s = load_w("wk_s", wk, BF16)
            wq_s = load_w("wq_s", wq, BF16)
            wv_s = load_w("wv_s", wv, BF16)
            bk_f = w_pool.tile([1, C], F32, name="bk_f")
            nc.sync.dma_start(bk_f, bk[None, :])
            bq_f = w_pool.tile([1, C], F32, name="bq_f")
            nc.sync.dma_start(bq_f, bq[None, :])
            bv_f = w_pool.tile([1, C], F32, name="bv_f")
            nc.sync.dma_start(bv_f, bv[None, :])
            btmps = [
                w_pool.tile([1, C], F32, name=f"btmp{i}") for i in range(3)
            ]
            bk2_p = w_pool.tile([P, CT], F32, name="bk2_p")
            bq2_p = w_pool.tile([P, CT], F32, name="bq2_p")
            bv2_p8 = w_pool.tile([P, CT], F8, name="bv2_p8")
            bv2_tmp = w_pool.tile([P, CT], F32, name="bv2_tmp")

            ps2 = tc.alloc_tile_pool(name=f"ps2{sfx}", bufs=6, space="PSUM")

            def fold_w(w_s, wf8, bias_f, dram_row, part_out, row_out):
                # bias' = b^T W + bias, computed before W is cast
                psb = ps2.tile([1, C], F32, tag="bias", name="psb", bufs=2)
                for ct in range(CT):
                    nc.tensor.matmul(
                        psb, lhsT=b_pr[:, ct : ct + 1], rhs=w_s[:, ct, :],
                        start=(ct == 0), stop=(ct == CT - 1),
                    )
                btmp = btmps[dram_row]
                nc.vector.tensor_tensor(btmp, psb, bias_f, ALU.add)
                if part_out is not None:
                    nc.sync.dma_start(
                        bias_dram[dram_row : dram_row + 1, :], btmp
                    )
                    nc.sync.dma_start(
                        part_out,
                        bias_dram[dram_row, :].rearrange("(j p) -> p j", p=P),
                    )
                if row_out is not None:
                    nc.vector.tensor_copy(row_out, btmp)
                # Wf8 = fp8(a * W) (rows scaled per input channel); wkf on
                # DVE, wqf on ScalarE (idle pre-exp), wvf on Pool - all three
                # fold chains run in parallel
                for ct in range(CT):
                    if dram_row == 2:
                        nc.gpsimd.tensor_scalar_mul(
                            wf8[:, ct, :], w_s[:, ct, :], a_p[:, ct : ct + 1]
                        )
                    elif (ct < 2) == (dram_row == 0):
                        nc.vector.tensor_scalar_mul(
                            wf8[:, ct, :], w_s[:, ct, :], a_p[:, ct : ct + 1]
                        )
                    else:
                        nc.scalar.activation(
                            wf8[:, ct, :], w_s[:, ct, :], AF.Identity,
                            scale=a_p[:, ct : ct + 1],
                        )

            wkf = wf_pool.tile([P, CT, C], F8, name="wkf")
            wqf = wf_pool.tile([P, CT, C], F8, name="wqf")
            wvf = wf_pool.tile([P, CT, C], F8, name="wvf")
            wp_s = wf_pool.tile([P, CT, C], F8, name="wp_s")
            nc.sync.dma_start(wp_s, wp.rearrange("(ko ki) n -> ki ko n", ki=P))
            fold_w(wk_s, wkf, bk_f, 0, bk2_p, None)

            # K^T[c_out, n]: stationary = wkf pair-slices, moving = xkvT.
            # nb-major so the casts S(qb=0) needs land first.
            NB2 = N // 512  # 8 pairs of 256-token chunks
            for nb in range(NB2):
                for co in range(CT):
                    ps = ps2.tile([P, 512], F32, tag="mm", name="ps")
                    for h in range(2):
                        for pr in range(2):
                            nc.tensor.matmul(
                                ps[:, h * 256 : (h + 1) * 256],
                                lhsT=wkf[:, 2 * pr : 2 * pr + 2, co * P : (co + 1) * P],
                                rhs=xkvT[:, 2 * pr : 2 * pr + 2,
                                         nb * 512 + h * 256 : nb * 512 + (h + 1) * 256],
                                start=(pr == 0), stop=(pr == 1),
                                perf_mode=DR,
                            )
                    if co < 2:
                        nc.vector.tensor_scalar_add(
                            kT[:, co, nb * 512 : (nb + 1) * 512],
                            ps, bk2_p[:, co : co + 1],
                        )
                    else:
                        nc.scalar.activation(
                            kT[:, co, nb * 512 : (nb + 1) * 512], ps,
                            AF.Identity, bias=bk2_p[:, co : co + 1],
                        )
            # wq/wv folds deferred here so ScalarE's K casts start sooner
            fold_w(wq_s, wqf, bq_f, 1, bq2_p, None)
            fold_w(wv_s, wvf, bv_f, 2, bv2_tmp, None)
            nc.vector.tensor_copy(bv2_p8, bv2_tmp)
            # Q^T[c_out, q]
            NBQ2 = NQ // 512  # 4
            for nb in range(NBQ2):
                for co in range(CT):
                    ps = ps2.tile([P, 512], F32, tag="mm", name="ps")
                    for h in range(2):
                        for pr in range(2):
                            nc.tensor.matmul(
                                ps[:, h * 256 : (h + 1) * 256],
                                lhsT=wqf[:, 2 * pr : 2 * pr + 2, co * P : (co + 1) * P],
                                rhs=xkvT[:, 2 * pr : 2 * pr + 2,
                                         nb * 512 + h * 256 : nb * 512 + (h + 1) * 256],
                                start=(pr == 0), stop=(pr == 1),
                                perf_mode=DR,
                            )
                    if co < 2:
                        nc.vector.tensor_scalar_add(
                            qT[:, co, nb * 512 : (nb + 1) * 512],
                            ps, bq2_p[:, co : co + 1],
                        )
                    else:
                        nc.scalar.activation(
                            qT[:, co, nb * 512 : (nb + 1) * 512], ps,
                            AF.Identity, bias=bq2_p[:, co : co + 1],
                        )
            # V[n, c_out] (keys on partitions); cast is a pure ScalarE copy
            # (bv2's contribution rides in bp_row instead)
            for kt in range(NT_KV):
                ps = ps2.tile([P, 512], F32, tag="mm", name="ps")
                for h in range(2):
                    for pr in range(2):
                        nc.tensor.matmul(
                            ps[:, h * 256 : (h + 1) * 256],
                            lhsT=xkvT[:, 2 * pr : 2 * pr + 2, kt * P : (kt + 1) * P],
                            rhs=wvf[:, 2 * pr : 2 * pr + 2, h * 256 : (h + 1) * 256],
                            start=(pr == 0), stop=(pr == 1),
                            perf_mode=DR,
                        )
                if kt % 2 == 0:
                    nc.scalar.activation(v_sb[:, kt, :], ps, AF.Copy)
                else:
                    nc.vector.tensor_copy(v_sb[:, kt, :], ps)
            # bp_row = bf16(bp + bv2 @ Wp)
            psb2 = ps2.tile([1, C], F32, tag="bias", name="psb", bufs=2)
            for ct in range(CT):
                nc.tensor.matmul(
                    psb2, lhsT=bv2_p8[:, ct : ct + 1], rhs=wp_s[:, ct, :],
                    start=(ct == 0), stop=(ct == CT - 1),
                )
            bp_sum = w_pool.tile([1, C], F32, name="bp_sum")
            nc.vector.tensor_tensor(bp_sum, psb2, bp_f, ALU.add)
            nc.vector.tensor_copy(bp_row, bp_sum)
            ps2.release()
            w_pool.release()
            free_xkvT()

            # ---- phase 3: attention, software-pipelined over q-blocks ----

            att = tc.alloc_tile_pool(name=f"att{sfx}", bufs=2, side="left")
            otp = tc.alloc_tile_pool(name=f"otp{sfx}", bufs=2, side="left")
            ps_s_pool = tc.alloc_tile_pool(name=f"ps_s{sfx}", bufs=2, space="PSUM")
            ps_o_pool = tc.alloc_tile_pool(name=f"ps_o{sfx}", bufs=1, space="PSUM")
            ps_y_pool = tc.alloc_tile_pool(name=f"ps_y{sfx}", bufs=1, space="PSUM")
            ps_d_pool = tc.alloc_tile_pool(name=f"ps_d{sfx}", bufs=1, space="PSUM")

            eT = {}
            ps_o = {}
            ps_d = {}
            oT = {}
            rd_b = {}

            def emit_S(qb):
                # S^T then exp into eT[qb]; ps_s holds 4 key tiles, the exp
                # activation covers all 4 in one go
                eT[qb] = att.tile([P, NT_KV, QB], F8, tag="eT", name="eT")
                ps_s = None
                for kp in range(KP):
                    if kp % 2 == 0:
                        ps_s = ps_s_pool.tile([P, 4, QB], F32, tag="s", name="ps_s")
                    for t in range(2):
                        kt = 2 * kp + t
                        for pr in range(2):
                            nc.tensor.matmul(
                                ps_s[:, 2 * (kp % 2) + t, :],
                                lhsT=kT[:, 2 * pr : 2 * pr + 2, kt * P : (kt + 1) * P],
                                rhs=qT[:, 2 * pr : 2 * pr + 2, qb * QB : (qb + 1) * QB],
                                start=(pr == 0), stop=(pr == 1),
                                perf_mode=DR,
                            )
                    if kp % 2 == 1:
                        nc.scalar.activation(
                            eT[qb][:, 2 * kp - 2 : 2 * kp + 2, :], ps_s,
                            AF.Exp, bias=negc, scale=SCALE2,
                        )
                    yield

            def emit_PV(qb, d_first=False):
                # O^T = V^T E^T and d = ones^T E^T, accumulated over key
                # pairs. HW gotcha (measured): round-robin interleaving of
                # several open DoubleRow accumulation chains corrupts the
                # results, so each cc's 16-step chain runs consecutively
                # (cc-outer); only the all-ones d chain (insensitive to the
                # weight-path hazard) plus immediate S groups may interleave.
                ps_o[qb] = ps_o_pool.tile([P, CT, QB], F32, tag="o", name="ps_o")
                ps_d[qb] = ps_d_pool.tile([1, QB], F32, tag="d", name="ps_d")
                if d_first:
                    # tail: run the whole d chain first so the 1/d roundtrip
                    # overlaps the cc chains
                    for i in range(KP):
                        nc.tensor.matmul(
                            ps_d[qb], lhsT=ones8,
                            rhs=eT[qb][:, 2 * i : 2 * i + 2, :],
                            start=(i == 0), stop=(i == KP - 1),
                            perf_mode=DR,
                        )
                for i in range(KP):
                    cc = i // 4
                    for j in range(4):
                        kp = 4 * (i % 4) + j
                        nc.tensor.matmul(
                            ps_o[qb][:, cc, :],
                            lhsT=v_sb[:, 2 * kp : 2 * kp + 2, cc * P : (cc + 1) * P],
                            rhs=eT[qb][:, 2 * kp : 2 * kp + 2, :],
                            start=(kp == 0), stop=(kp == KP - 1),
                            perf_mode=DR,
                        )
                    if not d_first and i < 4:
                        # d chain front-loaded (4 steps per slot) so the 1/d
                        # DRAM roundtrip overlaps the remaining cc chains
                        for j in range(4):
                            kd = 4 * i + j
                            nc.tensor.matmul(
                                ps_d[qb], lhsT=ones8,
                                rhs=eT[qb][:, 2 * kd : 2 * kd + 2, :],
                                start=(kd == 0), stop=(kd == KP - 1),
                                perf_mode=DR,
                            )
                    yield

            def emit_debug_dumps():
                nc.sync.dma_start(kT_dbg, kT[:, :, :])
                nc.sync.dma_start(qT_dbg, qT[:, :, :])
                nc.sync.dma_start(v_dbg, v_sb[:, :, :])

            def emit_rd(qb):
                # 1/d broadcast to all partitions via DRAM round trip
                dinv = stream.tile([1, QB], F32, tag="dinv", name="dinv", bufs=2)
                nc.vector.reciprocal(dinv, ps_d[qb])
                nc.sync.dma_start(rd_dram[qb : qb + 1, :], dinv)
                rd_b[qb] = stream.tile([P, 2, QB], F32, tag="rdb", name="rd_b", bufs=2)
                for rep in range(2):
                    nc.sync.dma_start(
                        rd_b[qb][:, rep, :], bcast_ap(rd_dram[qb, :], P)
                    )
                return dinv

            def emit_rd_oT(qb, dinv=None):
                # O^T normalized by 1/d during the fp8 cast
                if dinv is None:
                    dinv = emit_rd(qb)
                oT[qb] = otp.tile([P, CT, QB], F8, tag="oT", name="oT")
                for half in range(2):
                    nc.vector.tensor_tensor(
                        oT[qb][:, 2 * half : 2 * half + 2, :],
                        ps_o[qb][:, 2 * half : 2 * half + 2, :],
                        rd_b[qb], ALU.mult,
                    )
                if debug and qb == 0:
                    nc.sync.dma_start(e_dbg, eT[qb][:, :, :])
                    nc.sync.dma_start(d_dbg, dinv)
                    nc.sync.dma_start(o_dbg, oT[qb][:, :, :])
                    oraw_s = stream.tile([P, CT, QB], F32, tag="oraw", name="oraw", bufs=1)
                    nc.vector.tensor_copy(oraw_s, ps_o[qb][:, :, :])
                    nc.sync.dma_start(oraw_dbg, oraw_s)
                del ps_d[qb], ps_o[qb]

            def emit_proj_qc(qb, qc):
                # Y[q, c_out] = O^T-pair stationary @ wp, bp seeded via matmul
                ps_y = ps_y_pool.tile([P, C], F32, tag="y", name="ps_y")
                nc.tensor.matmul(
                    ps_y, lhsT=ones_bf, rhs=bp_row, start=True, stop=False,
                )
                for ch in range(2):
                    for pr in range(2):
                        nc.tensor.matmul(
                            ps_y[:, ch * 256 : (ch + 1) * 256],
                            lhsT=oT[qb][:, 2 * pr : 2 * pr + 2,
                                        qc * P : (qc + 1) * P],
                            rhs=wp_s[:, 2 * pr : 2 * pr + 2,
                                     ch * 256 : (ch + 1) * 256],
                            start=False, stop=(pr == 1),
                            perf_mode=DR,
                        )
                row0 = qb * QB + qc * P
                rt = stream.tile([P, C], F32, tag="rt", name="rt", bufs=4)
                nc.sync.dma_start(rt, x_res[row0 : row0 + P, :])
                ot = stream.tile([P, C], F32, tag="ot", name="ot", bufs=4)
                nc.vector.tensor_tensor(ot, ps_y, rt, ALU.add)
                nc.sync.dma_start(out[row0 : row0 + P, :], ot)
                if qc == 1:
                    del oT[qb], rd_b[qb]

            if debug:
                emit_debug_dumps()
            # pipeline: S(qb) interleaved per slot with PV(qb-1)'s chains;
            # proj(qb-2) halves slotted at cc-chain boundaries (after slot 7
            # only the hazard-immune d chain is open)
            for qb in range(QBN):
                sgen = emit_S(qb)
                pgen = emit_PV(qb - 1) if qb >= 1 else None
                for i in range(KP):
                    next(sgen)
                    if pgen is not None:
                        next(pgen)
                    if qb >= 2 and i == 7:
                        emit_proj_qc(qb - 2, 0)
                if qb >= 1:
                    emit_rd_oT(qb - 1)
                if qb >= 2:
                    emit_proj_qc(qb - 2, 1)
            gtail = emit_PV(QBN - 1, d_first=True)
            dinv_t = None
            for i in range(KP):
                next(gtail)
                if i == 0:
                    dinv_t = emit_rd(QBN - 1)
                elif i == 7:
                    emit_proj_qc(QBN - 2, 0)
                elif i == 11:
                    emit_proj_qc(QBN - 2, 1)
            emit_rd_oT(QBN - 1, dinv_t)
            emit_proj_qc(QBN - 1, 0)
            emit_proj_qc(QBN - 1, 1)

            ps_d_pool.release()
            ps_y_pool.release()
            ps_o_pool.release()
            ps_s_pool.release()
            otp.release()
            att.release()
            wf_pool.release()
            free_v()
            free_qT()
            free_kT()
            small.release()
            stream.release()
            consts.release()
            dscratch.release()

        for _it in range(iters):
            emit_body(f"_{_it}" if iters > 1 else "")

    _split_excess_waits(nc)
    return nc


_NC_CACHE = None


def get_nc():
    global _NC_CACHE
    if _NC_CACHE is None:
        _NC_CACHE = build_nc()
    return _NC_CACHE


def make_in_maps(inputs):
    import ml_dtypes

    hs = np.ascontiguousarray(np.asarray(inputs["hidden_states"], dtype=np.float32))
    x = hs.reshape(B, N, C)
    ws = {
        k: np.ascontiguousarray(np.asarray(inputs[k], dtype=np.float32))
        for k in ("Wq", "Wk", "Wv", "Wp", "bq", "bk", "bv", "bp",
                  "gn_scale", "gn_bias")
    }
    gmask = np.zeros((P, G // CT), np.float32)
    for p in range(P):
        gmask[p, p // GS] = 1.0
    part = lambda v: np.ascontiguousarray(v.reshape(CT, P).T)
    f8 = lambda v: np.ascontiguousarray(v).astype(ml_dtypes.float8_e4m3)
    bf = lambda v: np.ascontiguousarray(v).astype(ml_dtypes.bfloat16)
    common = {
        "wq": bf(ws["Wq"]), "wk": bf(ws["Wk"]), "wv": bf(ws["Wv"]),
        "wp": f8(ws["Wp"]),
        "bq": ws["bq"], "bk": ws["bk"], "bv": ws["bv"], "bp": ws["bp"],
        "gmask": gmask, "gmask2": np.ascontiguousarray(gmask.T),
        "gns_p": part(ws["gn_scale"]), "gnb_p": part(ws["gn_bias"]),
    }
    in_maps = []
    for core in range(8):
        b, h = divmod(core, 2)
        xb = x[b] if h == 0 else np.roll(x[b], -NQ, axis=0)
        in_maps.append({
            "xT": f8(xb.T),
            "x_res": np.ascontiguousarray(xb[:NQ]),
            **common,
        })
    return in_maps


def run(inputs, trace=False):
    from concourse.bass_utils import run_bass_kernel_spmd

    res = run_bass_kernel_spmd(
        get_nc(), make_in_maps(inputs), list(range(8)), trace=trace
    )
    out = np.empty((B, N, C), np.float32)
    for core in range(8):
        b, h = divmod(core, 2)
        out[b, h * NQ : (h + 1) * NQ] = res.results[core]["out"]
    return out.reshape(B, HH, WW, C), res

def kernel(**inputs) -> np.ndarray:
    out, _ = run(inputs)
    return out
